# revision 11
# baseline (speedup 1.0000x reference)
"""DANAttention Trainium2 kernel (8-core SPMD).

Layout strategy (per core): data-parallel over B (4 cores per batch) and
tensor-parallel over the 12 heads (3 heads per core).

All on-device tensors keep the "transposed" orientation that the PE's
partition-contracted matmul wants:
  - qT/kT: [head_dim(64) on partitions, tokens on free]  (from projection)
  - v:     [tokens on partitions, head_dim+1 on free]    (ones col -> softmax Z)
  - scoresT/attnT: [keys on partitions, queries on free]
  - outT partial: [out-features on partitions, tokens on free]

attn is written to DRAM per-head in [k, q] layout; the host returns a
zero-copy swapaxes view. out partials are summed on host (4 per batch).
"""

import numpy as np

import concourse.bass as bass
import concourse.mybir as mybir
import concourse.tile as tile
from concourse import mybir as _mybir
from concourse.vector_clock import ScopedClock
from concourse.bass_utils import run_bass_kernel_spmd

# Problem constants (hardcoded per harness contract).
B, S, E, H, TB = 2, 2048, 768, 12, 128
D = E // H            # 64
HL = 3                # heads per core
P = 128
KO = E // P           # 6  contraction chunks for projections
KK = S // P           # 16 key chunks
NQ = 512              # query chunk (psum free dim)
QC = S // NQ          # 4
NCORES = 8
MASK_NEG = 1.0e30
SCALE = 1.0 / (D ** 0.5)

F32 = mybir.dt.float32
F32R = mybir.dt.float32r

USE_F32R = True  # PE fp32 is 4 cycles/row; f32r is full-rate at N>=256.


class SplitDrainTileContext(tile.TileContext):
    """Tail drain in this walrus build allows only ONE sync-wait per CTRL
    instruction; split the drain's waits across preceding in-order SP NOPs."""

    def _drain_and_barrier(self, tick_clock, wait_clock):
        probe = self.nc.sync.nop(nofuse=True, hint="drain_wait_carrier")
        wait_clock.add_sem_waits(
            probe.ins, ScopedClock({None: tick_clock.global_clock})
        )
        si = probe.ins.sync_info
        waits = list(si.on_wait) if si and si.on_wait else []
        if len(waits) > 1:
            si.on_wait = waits[:1]
            for w in waits[1:]:
                nop = self.nc.sync.nop(nofuse=True, hint="drain_wait_carrier")
                nop.ins.sync_info = _mybir.SyncInfo(on_wait=[w], on_update=[])

        self.nc.sync.drain()
        self.nc.all_engine_barrier()
        assert self.sems is not None
        popped = self.nc._tile_sem_poison_stack.pop()
        assert popped is self._sem_poison
        self.nc.clear_and_free_semaphores(list(self.sems.allocated().values()))
        self.nc.all_engine_barrier()





def _split_multi_waits(nc):
    """This walrus build allows only ONE sync-wait command per instruction.
    Rewrite the serialized BIR: for any instruction carrying k>1 waits, hoist
    k-1 of them onto same-engine NoOps inserted immediately before it (engine
    streams are in-order, so this is semantically identical)."""
    import json

    m = json.loads(nc.to_json_bytes())
    counter = [0]

    def nop_for(inst, wait):
        counter[0] += 1
        return {
            "debug": inst.get("debug"),
            "engine": inst["engine"],
            "ins": [],
            "name": f"I-wsplit-{counter[0]}",
            "opcode": "NoOp",
            "outs": [],
            "sync_info": {"on_update": [], "on_wait": [wait]},
        }

    n_split = 0
    for f in m["functions"]:
        for blk in f["blocks"]:
            out = []
            for inst in blk["instructions"]:
                si = inst.get("sync_info") or {}
                waits = si.get("on_wait") or []
                if len(waits) > 1:
                    n_split += 1
                    for w in waits[:-1]:
                        out.append(nop_for(inst, w))
                    si["on_wait"] = waits[-1:]
                out.append(inst)
            blk["instructions"] = out

    patched_bytes = json.dumps(m).encode()
    nc.to_json_bytes = lambda: patched_bytes
    nc.to_json_str = lambda: patched_bytes.decode()
    nc.to_json = lambda: json.loads(patched_bytes)
    return n_split


def _bcast_ap(dram_ap, parts):
    """DRAM AP replicated across `parts` partitions (partition step 0)."""
    return bass.AP(
        tensor=dram_ap.tensor,
        offset=dram_ap.offset,
        ap=[[0, parts]] + [list(x) for x in dram_ap.ap],
    )


def build_nc():
    nc = bass.Bass()

    xq = nc.dram_tensor("xq_t", [E, S], F32R, kind="ExternalInput")
    xk = nc.dram_tensor("xk_t", [E, S], F32R, kind="ExternalInput")
    xv = nc.dram_tensor("xv_t", [E, S], F32R, kind="ExternalInput")
    wq = nc.dram_tensor("wq_t", [E, HL * D], F32R, kind="ExternalInput")
    wk = nc.dram_tensor("wk_t", [E, HL * D], F32R, kind="ExternalInput")
    wv = nc.dram_tensor("wv_t", [E, HL * D], F32R, kind="ExternalInput")
    wo = nc.dram_tensor("wo_t", [D, HL, E], F32R, kind="ExternalInput")
    bq = nc.dram_tensor("bq2", [P, 2], F32, kind="ExternalInput")
    bk = nc.dram_tensor("bk2", [P, 2], F32, kind="ExternalInput")
    bv = nc.dram_tensor("bv_row", [HL * D], F32, kind="ExternalInput")
    bo = nc.dram_tensor("bo6", [P, KO], F32, kind="ExternalInput")
    taskb = nc.dram_tensor("task_b", [TB], F32, kind="ExternalInput")
    wtb = nc.dram_tensor("wtb_t", [TB, HL], F32, kind="ExternalInput")
    btb = nc.dram_tensor("btb3", [HL], F32, kind="ExternalInput")
    maskb = nc.dram_tensor("maskbias", [P, KK], F32, kind="ExternalInput")

    attn_t = nc.dram_tensor("attn_t", [HL, S, S], F32R, kind="ExternalOutput")
    out_t = nc.dram_tensor("out_t", [E, S], F32, kind="ExternalOutput")
    tb_dram = nc.dram_tensor("tb_scratch", [HL], F32)

    with SplitDrainTileContext(nc) as tc:
        with (
            tc.tile_pool(name="singles", bufs=1) as singles,
            tc.tile_pool(name="xp", bufs=2) as xp,
            tc.tile_pool(name="psum", bufs=1, space="PSUM") as psum,
        ):
            # ---- long-lived tiles ----
            wo_sb = singles.tile([D, HL, E], F32R)
            nc.sync.dma_start(out=wo_sb, in_=wo[:])
            bo_sb = singles.tile([P, KO], F32)
            nc.sync.dma_start(out=bo_sb, in_=bo[:])
            expb = singles.tile([P, HL, KK], F32)
            # v with ones column (for softmax denominator via PV matmul)
            v_sb = singles.tile([P, KK, HL, D + 1], F32R)
            nc.vector.memset(v_sb[:, :, :, D].bitcast(F32), 1.0)
            # qT/kT: [heads*64 on partitions, tokens on free]
            qt0 = singles.tile([P, S], F32R)   # heads 0,1
            qt1 = singles.tile([D, S], F32R)   # head 2
            kt0 = singles.tile([P, S], F32R)
            kt1 = singles.tile([D, S], F32R)
            # ones row used to broadcast 1/Z across partitions via outer product
            onesk = singles.tile([D + 1, P], F32)
            nc.vector.memset(onesk[D : D + 1, :], 1.0)

            with tc.tile_pool(name="wpool", bufs=1) as wpool:
                # ---- projection weights / biases (phase A only) ----
                wq_sb = wpool.tile([P, KO, HL * D], F32R)
                nc.sync.dma_start(
                    out=wq_sb, in_=wq[:].rearrange("(ko p) m -> p ko m", p=P)
                )
                wk_sb = wpool.tile([P, KO, HL * D], F32R)
                nc.sync.dma_start(
                    out=wk_sb, in_=wk[:].rearrange("(ko p) m -> p ko m", p=P)
                )
                wv_sb = wpool.tile([P, KO, HL * D], F32R)
                nc.sync.dma_start(
                    out=wv_sb, in_=wv[:].rearrange("(ko p) m -> p ko m", p=P)
                )
                bq_sb = wpool.tile([P, 2], F32)
                nc.sync.dma_start(out=bq_sb, in_=bq[:])
                bk_sb = wpool.tile([P, 2], F32)
                nc.sync.dma_start(out=bk_sb, in_=bk[:])
                bv_sb = wpool.tile([P, HL * D], F32)
                nc.sync.dma_start(out=bv_sb, in_=_bcast_ap(bv[:], P))
                mb_sb = wpool.tile([P, KK], F32)
                nc.sync.dma_start(out=mb_sb, in_=maskb[:])

                # ---- task bias: tb[h] = task_bias @ Wtb[h].T + btb[h] ----
                task_sb = wpool.tile([TB, 1], F32)
                nc.sync.dma_start(out=task_sb, in_=taskb[:, None])
                wtb_sb = wpool.tile([TB, HL], F32)
                nc.sync.dma_start(out=wtb_sb, in_=wtb[:])
                btb_sb = wpool.tile([HL, 1], F32)
                nc.sync.dma_start(out=btb_sb, in_=btb[:, None])
                ps_tb = psum.tile([HL, 1], F32, tag="ctx", bufs=2)
                nc.tensor.matmul(
                    ps_tb, lhsT=wtb_sb, rhs=task_sb, start=True, stop=True
                )
                tb_sb = wpool.tile([HL, 1], F32)
                nc.scalar.activation(
                    out=tb_sb, in_=ps_tb,
                    func=mybir.ActivationFunctionType.Identity,
                    bias=btb_sb, scale=1.0,
                )
                # broadcast tb to all partitions via DRAM bounce
                nc.sync.dma_start(out=tb_dram[:, None], in_=tb_sb)
                tb_bc = wpool.tile([P, HL], F32)
                nc.sync.dma_start(out=tb_bc, in_=_bcast_ap(tb_dram[:], P))
                # per-head exp bias [128, h, kk] = maskbias + tb[h]
                for h in range(HL):
                    nc.vector.tensor_scalar_add(
                        out=expb[:, h, :], in0=mb_sb, scalar1=tb_bc[:, h : h + 1]
                    )

                # ---- q/k projections (chunked over tokens) ----
                for x_dram, w_sb, b_sb, t0, t1 in (
                    (xq, wq_sb, bq_sb, qt0, qt1),
                    (xk, wk_sb, bk_sb, kt0, kt1),
                ):
                    x_re = x_dram[:].rearrange("(ko p) q -> p ko q", p=P)
                    for qc in range(QC):
                        x_sb = xp.tile([P, KO, NQ], F32R, tag="xT")
                        nc.sync.dma_start(
                            out=x_sb, in_=x_re[:, :, qc * NQ : (qc + 1) * NQ]
                        )
                        for mt in range(2):
                            msz = P if mt == 0 else D
                            dst = t0 if mt == 0 else t1
                            ps = psum.tile([P, NQ], F32, tag="mm_out", bufs=2)
                            for ko in range(KO):
                                nc.tensor.matmul(
                                    ps[:msz],
                                    lhsT=w_sb[:, ko, mt * P : mt * P + msz]
                                    ,
                                    rhs=x_sb[:, ko, :],
                                    start=(ko == 0),
                                    stop=(ko == KO - 1),
                                )
                            nc.scalar.activation(
                                out=dst[:msz, qc * NQ : (qc + 1) * NQ],
                                in_=ps[:msz],
                                func=mybir.ActivationFunctionType.Identity,
                                bias=b_sb[:msz, mt : mt + 1],
                                scale=1.0,
                            )

                # ---- v projection: [tokens on partitions, heads x 64] ----
                xv_re = xv[:].rearrange("(ko p) q -> p ko q", p=P)
                for qc in range(QC):
                    xv_sb = xp.tile([P, KO, NQ], F32R, tag="xT")
                    nc.sync.dma_start(
                        out=xv_sb, in_=xv_re[:, :, qc * NQ : (qc + 1) * NQ]
                    )
                    for ml in range(NQ // P):
                        m = qc * (NQ // P) + ml
                        ps = psum.tile([P, NQ], F32, tag="mm_out", bufs=2)
                        for ko in range(KO):
                            nc.tensor.matmul(
                                ps[:, : HL * D],
                                lhsT=xv_sb[:, ko, ml * P : (ml + 1) * P]
                                ,
                                rhs=wv_sb[:, ko, :],
                                start=(ko == 0),
                                stop=(ko == KO - 1),
                            )
                        nc.vector.tensor_tensor(
                            v_sb[:, m, :, 0:D],
                            ps[:, : HL * D].rearrange("p (h d) -> p h d", h=HL),
                            bv_sb.rearrange("p (h d) -> p h d", h=HL),
                            mybir.AluOpType.add,
                        )

            # ---- attention ----
            attn_dst = attn_t[:].rearrange("h (kk p) q -> h p kk q", p=P)
            out_dst = out_t[:].rearrange("(io p) q -> p io q", p=P)

            with (
                tc.tile_pool(name="attnp", bufs=2) as attnp,
                tc.tile_pool(name="ctxp", bufs=2) as ctxp,
                tc.tile_pool(name="outp", bufs=2) as outp,
                tc.tile_pool(name="rzp", bufs=2) as rzp,
            ):
              for qc in range(QC):
                qsl = slice(qc * NQ, (qc + 1) * NQ)
                ctx_tiles = []
                for h in range(HL):
                    if h < 2:
                        qt_h = qt0[h * D : (h + 1) * D, qsl]
                        kt_h = kt0[h * D : (h + 1) * D, :]
                    else:
                        qt_h = qt1[:, qsl]
                        kt_h = kt1[:, :]

                    e_sb = attnp.tile([P, KK, NQ], F32R, tag="expT")
                    for kk in range(KK):
                        ps_s = psum.tile([P, NQ], F32, tag="scores", bufs=2)
                        nc.tensor.matmul(
                            ps_s,
                            lhsT=kt_h[:, kk * P : (kk + 1) * P],
                            rhs=qt_h,
                            start=True,
                            stop=True,
                        )
                        nc.scalar.activation(
                            out=e_sb[:, kk, :],
                            in_=ps_s,
                            func=mybir.ActivationFunctionType.Exp,
                            bias=expb[:, h, kk : kk + 1],
                            scale=SCALE,
                        )

                    ps_c = psum.tile([D + 1, NQ], F32, tag="ctx", bufs=2)
                    for kk in range(KK):
                        nc.tensor.matmul(
                            ps_c,
                            lhsT=v_sb[:, kk, h, :],
                            rhs=e_sb[:, kk, :],
                            start=(kk == 0),
                            stop=(kk == KK - 1),
                        )

                    # Z row -> 1/Z -> broadcast to all partitions (outer product)
                    rz = rzp.tile([D + 1, NQ], F32, tag="rz")
                    nc.vector.reciprocal(out=rz[D : D + 1, :], in_=ps_c[D : D + 1, :])
                    ps_b = psum.tile([P, NQ], F32, tag="bcast", bufs=2)
                    nc.tensor.matmul(
                        ps_b,
                        lhsT=onesk[D : D + 1, :],
                        rhs=rz[D : D + 1, :],
                        start=True,
                        stop=True,
                    )
                    rzb = rzp.tile([P, NQ], F32, tag="rzb")
                    nc.vector.tensor_copy(out=rzb, in_=ps_b)

                    # normalize attention (in place) and ctx
                    nc.vector.tensor_tensor(
                        e_sb[:, :, :],
                        e_sb[:, :, :],
                        rzb[:, None, :].to_broadcast([P, KK, NQ]),
                        mybir.AluOpType.mult,
                    )
                    ctx_h = ctxp.tile([D, NQ], F32R, tag=f"ctx{h}")
                    nc.vector.tensor_tensor(
                        ctx_h, ps_c[0:D, :], rzb[0:D, :], mybir.AluOpType.mult
                    )
                    ctx_tiles.append(ctx_h)

                    nc.sync.dma_start(out=attn_dst[h][:, :, qsl], in_=e_sb)

                # out projection partial for this q chunk
                o_sb = outp.tile([P, KO, NQ], F32, tag="out")
                for it in range(KO):
                    ps_o = psum.tile([P, NQ], F32, tag="mm_out", bufs=2)
                    for h in range(HL):
                        nc.tensor.matmul(
                            ps_o,
                            lhsT=wo_sb[:, h, it * P : (it + 1) * P],
                            rhs=ctx_tiles[h],
                            start=(h == 0),
                            stop=(h == HL - 1),
                        )
                    nc.scalar.activation(
                        out=o_sb[:, it, :],
                        in_=ps_o,
                        func=mybir.ActivationFunctionType.Identity,
                        bias=bo_sb[:, it : it + 1],
                        scale=1.0,
                    )
                nc.sync.dma_start(out=out_dst[:, :, qsl], in_=o_sb)

    return nc


_NC_CACHE = None


def _get_nc():
    global _NC_CACHE
    if _NC_CACHE is None:
        _NC_CACHE = build_nc()
        _split_multi_waits(_NC_CACHE)
    return _NC_CACHE


def _core_inputs(c, query, key, value, task_bias, attention_mask,
                 Wq, bq, Wk, bk, Wv, bv, Wtb, btb, Wo, bo, xts):
    b = c // 4
    g = c % 4
    hs = g * HL                      # first head of this core
    fs = slice(hs * D, (hs + HL) * D)  # feature slice (192 wide)

    def bias2(vec):
        out = np.zeros((P, 2), np.float32)
        sl = np.asarray(vec[fs], np.float32)
        out[:, 0] = sl[0:P]
        out[: HL * D - P, 1] = sl[P:]
        return out

    bo_eff = np.asarray(bo, np.float32) if g == 0 else np.zeros((E,), np.float32)
    m = np.asarray(attention_mask[b], np.float32)
    maskbias = np.ascontiguousarray(
        ((m - 1.0) * MASK_NEG).reshape(KK, P).T
    ).astype(np.float32)

    wo_slice = np.asarray(Wo[:, fs], np.float32)          # [768, 192]
    wo_t = np.ascontiguousarray(
        wo_slice.T.reshape(HL, D, E).transpose(1, 0, 2)
    )                                                      # [64, 3, 768]

    return {
        "xq_t": xts[("q", b)],
        "xk_t": xts[("k", b)],
        "xv_t": xts[("v", b)],
        "wq_t": np.ascontiguousarray(np.asarray(Wq, np.float32)[fs].T),
        "wk_t": np.ascontiguousarray(np.asarray(Wk, np.float32)[fs].T),
        "wv_t": np.ascontiguousarray(np.asarray(Wv, np.float32)[fs].T),
        "wo_t": wo_t,
        "bq2": bias2(bq),
        "bk2": bias2(bk),
        "bv_row": np.ascontiguousarray(np.asarray(bv, np.float32)[fs]),
        "bo6": np.ascontiguousarray(bo_eff.reshape(KO, P).T),
        "task_b": np.asarray(task_bias[b], np.float32),
        "wtb_t": np.ascontiguousarray(np.asarray(Wtb, np.float32)[hs : hs + HL].T),
        "btb3": np.ascontiguousarray(np.asarray(btb, np.float32)[hs : hs + HL]),
        "maskbias": maskbias,
    }


def kernel(query, key, value, task_bias, attention_mask,
           Wq, bq, Wk, bk, Wv, bv, Wtb, btb, Wo, bo, **run_kwargs):
    query = np.asarray(query, np.float32)
    key = np.asarray(key, np.float32)
    value = np.asarray(value, np.float32)

    # host-side shard prep: transposed activations, shared across cores of a batch
    xts = {}
    for b in range(B):
        xts[("q", b)] = np.ascontiguousarray(query[b].T)
        xts[("k", b)] = np.ascontiguousarray(key[b].T)
        xts[("v", b)] = np.ascontiguousarray(value[b].T)

    in_maps = [
        _core_inputs(c, query, key, value, task_bias, attention_mask,
                     Wq, bq, Wk, bk, Wv, bv, Wtb, btb, Wo, bo, xts)
        for c in range(NCORES)
    ]

    nc = _get_nc()
    res = run_bass_kernel_spmd(nc, in_maps, list(range(NCORES)), **run_kwargs)
    results = res.results

    # gather attn: per-core [3, k, q] -> [B, H, k, q] -> transpose view [B,H,q,k]
    attn_kq = np.empty((B, H, S, S), np.float32)
    for c in range(NCORES):
        b, hs = c // 4, (c % 4) * HL
        attn_kq[b, hs : hs + HL] = results[c]["attn_t"]
    attn = attn_kq.transpose(0, 1, 3, 2)

    # gather out: sum the 4 partials per batch, transpose to [S, E]
    out = np.empty((B, S, E), np.float32)
    for b in range(B):
        acc = results[4 * b]["out_t"].copy()
        for g in range(1, 4):
            acc += results[4 * b + g]["out_t"]
        out[b] = acc.T

    if run_kwargs:
        kernel.last_result = res
    return out, attn


# revision 13
# speedup vs baseline: 1.0008x; 1.0008x over previous
"""DANAttention Trainium2 kernel (8-core SPMD).

Layout strategy (per core): data-parallel over B (4 cores per batch) and
tensor-parallel over the 12 heads (3 heads per core).

All on-device tensors keep the "transposed" orientation that the PE's
partition-contracted matmul wants:
  - qT/kT: [head_dim(64) on partitions, tokens on free]  (from projection)
  - v:     [tokens on partitions, head_dim+1 on free]    (ones col -> softmax Z)
  - scoresT/attnT: [keys on partitions, queries on free]
  - outT partial: [out-features on partitions, tokens on free]

attn is written to DRAM per-head in [k, q] layout; the host returns a
zero-copy swapaxes view. out partials are summed on host (4 per batch).
"""

import numpy as np

import concourse.bass as bass
import concourse.mybir as mybir
import concourse.tile as tile
from concourse import mybir as _mybir
from concourse.vector_clock import ScopedClock
from concourse.bass_utils import run_bass_kernel_spmd

# Problem constants (hardcoded per harness contract).
B, S, E, H, TB = 2, 2048, 768, 12, 128
D = E // H            # 64
HL = 3                # heads per core
P = 128
KO = E // P           # 6  contraction chunks for projections
KK = S // P           # 16 key chunks
NQ = 512              # query chunk (psum free dim)
QC = S // NQ          # 4
NCORES = 8
MASK_NEG = 1.0e30
SCALE = 1.0 / (D ** 0.5)

F32 = mybir.dt.float32
F32R = mybir.dt.float32r

USE_F32R = True  # PE fp32 is 4 cycles/row; f32r is full-rate at N>=256.


class SplitDrainTileContext(tile.TileContext):
    """Tail drain in this walrus build allows only ONE sync-wait per CTRL
    instruction; split the drain's waits across preceding in-order SP NOPs."""

    def _drain_and_barrier(self, tick_clock, wait_clock):
        probe = self.nc.sync.nop(nofuse=True, hint="drain_wait_carrier")
        wait_clock.add_sem_waits(
            probe.ins, ScopedClock({None: tick_clock.global_clock})
        )
        si = probe.ins.sync_info
        waits = list(si.on_wait) if si and si.on_wait else []
        if len(waits) > 1:
            si.on_wait = waits[:1]
            for w in waits[1:]:
                nop = self.nc.sync.nop(nofuse=True, hint="drain_wait_carrier")
                nop.ins.sync_info = _mybir.SyncInfo(on_wait=[w], on_update=[])

        self.nc.sync.drain()
        self.nc.all_engine_barrier()
        assert self.sems is not None
        popped = self.nc._tile_sem_poison_stack.pop()
        assert popped is self._sem_poison
        self.nc.clear_and_free_semaphores(list(self.sems.allocated().values()))
        self.nc.all_engine_barrier()





def _split_multi_waits(nc):
    """This walrus build allows only ONE sync-wait command per instruction.
    Rewrite the serialized BIR: for any instruction carrying k>1 waits, hoist
    k-1 of them onto same-engine NoOps inserted immediately before it (engine
    streams are in-order, so this is semantically identical)."""
    import json

    m = json.loads(nc.to_json_bytes())
    counter = [0]

    def nop_for(inst, wait):
        counter[0] += 1
        return {
            "debug": inst.get("debug"),
            "engine": inst["engine"],
            "ins": [],
            "name": f"I-wsplit-{counter[0]}",
            "opcode": "NoOp",
            "outs": [],
            "sync_info": {"on_update": [], "on_wait": [wait]},
        }

    n_split = 0
    for f in m["functions"]:
        for blk in f["blocks"]:
            out = []
            for inst in blk["instructions"]:
                si = inst.get("sync_info") or {}
                waits = si.get("on_wait") or []
                if len(waits) > 1:
                    n_split += 1
                    for w in waits[:-1]:
                        out.append(nop_for(inst, w))
                    si["on_wait"] = waits[-1:]
                out.append(inst)
            blk["instructions"] = out

    patched_bytes = json.dumps(m).encode()
    nc.to_json_bytes = lambda: patched_bytes
    nc.to_json_str = lambda: patched_bytes.decode()
    nc.to_json = lambda: json.loads(patched_bytes)
    return n_split


def _bcast_ap(dram_ap, parts):
    """DRAM AP replicated across `parts` partitions (partition step 0)."""
    return bass.AP(
        tensor=dram_ap.tensor,
        offset=dram_ap.offset,
        ap=[[0, parts]] + [list(x) for x in dram_ap.ap],
    )


def build_nc():
    nc = bass.Bass()

    xq = nc.dram_tensor("xq_t", [E, S], F32R, kind="ExternalInput")
    xk = nc.dram_tensor("xk_t", [E, S], F32R, kind="ExternalInput")
    xv = nc.dram_tensor("xv_t", [E, S], F32R, kind="ExternalInput")
    wq = nc.dram_tensor("wq_t", [E, HL * D], F32R, kind="ExternalInput")
    wk = nc.dram_tensor("wk_t", [E, HL * D], F32R, kind="ExternalInput")
    wv = nc.dram_tensor("wv_t", [E, HL * D], F32R, kind="ExternalInput")
    wo = nc.dram_tensor("wo_t", [D, HL, E], F32R, kind="ExternalInput")
    bq = nc.dram_tensor("bq2", [P, 2], F32, kind="ExternalInput")
    bk = nc.dram_tensor("bk2", [P, 2], F32, kind="ExternalInput")
    bv = nc.dram_tensor("bv_row", [HL * D], F32, kind="ExternalInput")
    bo = nc.dram_tensor("bo6", [P, KO], F32, kind="ExternalInput")
    taskb = nc.dram_tensor("task_b", [TB], F32, kind="ExternalInput")
    wtb = nc.dram_tensor("wtb_t", [TB, HL], F32, kind="ExternalInput")
    btb = nc.dram_tensor("btb3", [HL], F32, kind="ExternalInput")
    maskb = nc.dram_tensor("maskbias", [P, KK], F32, kind="ExternalInput")

    attn_t = nc.dram_tensor("attn_t", [HL, S, S], F32R, kind="ExternalOutput")
    out_t = nc.dram_tensor("out_t", [E, S], F32, kind="ExternalOutput")
    tb_dram = nc.dram_tensor("tb_scratch", [HL], F32)

    with SplitDrainTileContext(nc) as tc:
        with (
            tc.tile_pool(name="singles", bufs=1) as singles,
            tc.tile_pool(name="xp", bufs=2) as xp,
            tc.tile_pool(name="psum", bufs=1, space="PSUM") as psum,
        ):
            # ---- long-lived tiles ----
            wo_sb = singles.tile([D, HL, E], F32R)
            nc.sync.dma_start(out=wo_sb, in_=wo[:])
            bo_sb = singles.tile([P, KO], F32)
            nc.sync.dma_start(out=bo_sb, in_=bo[:])
            expb = singles.tile([P, HL, KK], F32)
            # v with ones column (for softmax denominator via PV matmul)
            v_sb = singles.tile([P, KK, HL, D + 1], F32R)
            nc.vector.memset(v_sb[:, :, :, D].bitcast(F32), 1.0)
            # qT/kT: [heads*64 on partitions, tokens on free]
            qt0 = singles.tile([P, S], F32R)   # heads 0,1
            qt1 = singles.tile([D, S], F32R)   # head 2
            kt0 = singles.tile([P, S], F32R)
            kt1 = singles.tile([D, S], F32R)
            # ones row used to broadcast 1/Z across partitions via outer product
            onesk = singles.tile([D + 1, P], F32)
            nc.vector.memset(onesk[D : D + 1, :], 1.0)

            # HAM heater: f32r matmuls don't register as PE activity, so the
            # clock gate settles at 1.2 GHz mid-kernel. Sprinkle tiny bf16
            # matmuls through the PE stream to keep the 2.4 GHz clock.
            heat_w = singles.tile([P, D], mybir.dt.bfloat16)
            nc.vector.memset(heat_w, 0.0)
            ps_heat = psum.tile([D, D], F32, tag="heat", bufs=1)

            def _heat():
                nc.tensor.matmul(
                    ps_heat, lhsT=heat_w, rhs=heat_w,
                    start=True, stop=True, skip_group_check=True,
                )

            with tc.tile_pool(name="wpool", bufs=1) as wpool:
                # ---- projection weights / biases (phase A only) ----
                wq_sb = wpool.tile([P, KO, HL * D], F32R)
                nc.sync.dma_start(
                    out=wq_sb, in_=wq[:].rearrange("(ko p) m -> p ko m", p=P)
                )
                wk_sb = wpool.tile([P, KO, HL * D], F32R)
                nc.sync.dma_start(
                    out=wk_sb, in_=wk[:].rearrange("(ko p) m -> p ko m", p=P)
                )
                wv_sb = wpool.tile([P, KO, HL * D], F32R)
                nc.sync.dma_start(
                    out=wv_sb, in_=wv[:].rearrange("(ko p) m -> p ko m", p=P)
                )
                bq_sb = wpool.tile([P, 2], F32)
                nc.sync.dma_start(out=bq_sb, in_=bq[:])
                bk_sb = wpool.tile([P, 2], F32)
                nc.sync.dma_start(out=bk_sb, in_=bk[:])
                bv_sb = wpool.tile([P, HL * D], F32)
                nc.sync.dma_start(out=bv_sb, in_=_bcast_ap(bv[:], P))
                mb_sb = wpool.tile([P, KK], F32)
                nc.sync.dma_start(out=mb_sb, in_=maskb[:])

                # ---- task bias: tb[h] = task_bias @ Wtb[h].T + btb[h] ----
                task_sb = wpool.tile([TB, 1], F32)
                nc.sync.dma_start(out=task_sb, in_=taskb[:, None])
                wtb_sb = wpool.tile([TB, HL], F32)
                nc.sync.dma_start(out=wtb_sb, in_=wtb[:])
                btb_sb = wpool.tile([HL, 1], F32)
                nc.sync.dma_start(out=btb_sb, in_=btb[:, None])
                ps_tb = psum.tile([HL, 1], F32, tag="ctx", bufs=2)
                nc.tensor.matmul(
                    ps_tb, lhsT=wtb_sb, rhs=task_sb, start=True, stop=True
                )
                tb_sb = wpool.tile([HL, 1], F32)
                nc.scalar.activation(
                    out=tb_sb, in_=ps_tb,
                    func=mybir.ActivationFunctionType.Identity,
                    bias=btb_sb, scale=1.0,
                )
                # broadcast tb to all partitions via DRAM bounce
                nc.sync.dma_start(out=tb_dram[:, None], in_=tb_sb)
                tb_bc = wpool.tile([P, HL], F32)
                nc.sync.dma_start(out=tb_bc, in_=_bcast_ap(tb_dram[:], P))
                # per-head exp bias [128, h, kk] = maskbias + tb[h]
                for h in range(HL):
                    nc.vector.tensor_scalar_add(
                        out=expb[:, h, :], in0=mb_sb, scalar1=tb_bc[:, h : h + 1]
                    )

                # ---- q/k projections (chunked over tokens) ----
                for x_dram, w_sb, b_sb, t0, t1 in (
                    (xq, wq_sb, bq_sb, qt0, qt1),
                    (xk, wk_sb, bk_sb, kt0, kt1),
                ):
                    x_re = x_dram[:].rearrange("(ko p) q -> p ko q", p=P)
                    for qc in range(QC):
                        x_sb = xp.tile([P, KO, NQ], F32R, tag="xT")
                        nc.sync.dma_start(
                            out=x_sb, in_=x_re[:, :, qc * NQ : (qc + 1) * NQ]
                        )
                        for mt in range(2):
                            msz = P if mt == 0 else D
                            dst = t0 if mt == 0 else t1
                            _heat()
                            ps = psum.tile([P, NQ], F32, tag="mm_out", bufs=2)
                            for ko in range(KO):
                                nc.tensor.matmul(
                                    ps[:msz],
                                    lhsT=w_sb[:, ko, mt * P : mt * P + msz]
                                    ,
                                    rhs=x_sb[:, ko, :],
                                    start=(ko == 0),
                                    stop=(ko == KO - 1),
                                )
                            nc.scalar.activation(
                                out=dst[:msz, qc * NQ : (qc + 1) * NQ],
                                in_=ps[:msz],
                                func=mybir.ActivationFunctionType.Identity,
                                bias=b_sb[:msz, mt : mt + 1],
                                scale=1.0,
                            )

                # ---- v projection: [tokens on partitions, heads x 64] ----
                xv_re = xv[:].rearrange("(ko p) q -> p ko q", p=P)
                for qc in range(QC):
                    xv_sb = xp.tile([P, KO, NQ], F32R, tag="xT")
                    nc.sync.dma_start(
                        out=xv_sb, in_=xv_re[:, :, qc * NQ : (qc + 1) * NQ]
                    )
                    for ml in range(NQ // P):
                        m = qc * (NQ // P) + ml
                        _heat()
                        ps = psum.tile([P, NQ], F32, tag="mm_out", bufs=2)
                        for ko in range(KO):
                            nc.tensor.matmul(
                                ps[:, : HL * D],
                                lhsT=xv_sb[:, ko, ml * P : (ml + 1) * P]
                                ,
                                rhs=wv_sb[:, ko, :],
                                start=(ko == 0),
                                stop=(ko == KO - 1),
                            )
                        nc.vector.tensor_tensor(
                            v_sb[:, m, :, 0:D],
                            ps[:, : HL * D].rearrange("p (h d) -> p h d", h=HL),
                            bv_sb.rearrange("p (h d) -> p h d", h=HL),
                            mybir.AluOpType.add,
                        )

            # ---- attention ----
            attn_dst = attn_t[:].rearrange("h (kk p) q -> h p kk q", p=P)
            out_dst = out_t[:].rearrange("(io p) q -> p io q", p=P)

            with (
                tc.tile_pool(name="attnp", bufs=2) as attnp,
                tc.tile_pool(name="ctxp", bufs=2) as ctxp,
                tc.tile_pool(name="outp", bufs=2) as outp,
                tc.tile_pool(name="rzp", bufs=2) as rzp,
            ):
              for qc in range(QC):
                qsl = slice(qc * NQ, (qc + 1) * NQ)
                ctx_tiles = []
                for h in range(HL):
                    if h < 2:
                        qt_h = qt0[h * D : (h + 1) * D, qsl]
                        kt_h = kt0[h * D : (h + 1) * D, :]
                    else:
                        qt_h = qt1[:, qsl]
                        kt_h = kt1[:, :]

                    e_sb = attnp.tile([P, KK, NQ], F32R, tag="expT")
                    for kk in range(KK):
                        if kk % 5 == 0:
                            _heat()
                        ps_s = psum.tile([P, NQ], F32, tag="scores", bufs=2)
                        nc.tensor.matmul(
                            ps_s,
                            lhsT=kt_h[:, kk * P : (kk + 1) * P],
                            rhs=qt_h,
                            start=True,
                            stop=True,
                        )
                        nc.scalar.activation(
                            out=e_sb[:, kk, :],
                            in_=ps_s,
                            func=mybir.ActivationFunctionType.Exp,
                            bias=expb[:, h, kk : kk + 1],
                            scale=SCALE,
                        )

                    ps_c = psum.tile([D + 1, NQ], F32, tag="ctx", bufs=2)
                    for kk in range(KK):
                        if kk % 5 == 0:
                            _heat()
                        nc.tensor.matmul(
                            ps_c,
                            lhsT=v_sb[:, kk, h, :],
                            rhs=e_sb[:, kk, :],
                            start=(kk == 0),
                            stop=(kk == KK - 1),
                        )

                    # Z row -> 1/Z -> broadcast to all partitions (outer product)
                    rz = rzp.tile([D + 1, NQ], F32, tag="rz")
                    nc.vector.reciprocal(out=rz[D : D + 1, :], in_=ps_c[D : D + 1, :])
                    ps_b = psum.tile([P, NQ], F32, tag="bcast", bufs=1)
                    nc.tensor.matmul(
                        ps_b,
                        lhsT=onesk[D : D + 1, :],
                        rhs=rz[D : D + 1, :],
                        start=True,
                        stop=True,
                    )
                    rzb = rzp.tile([P, NQ], F32, tag="rzb")
                    nc.vector.tensor_copy(out=rzb, in_=ps_b)

                    # normalize attention (in place) and ctx
                    nc.vector.tensor_tensor(
                        e_sb[:, :, :],
                        e_sb[:, :, :],
                        rzb[:, None, :].to_broadcast([P, KK, NQ]),
                        mybir.AluOpType.mult,
                    )
                    ctx_h = ctxp.tile([D, NQ], F32R, tag=f"ctx{h}")
                    nc.vector.tensor_tensor(
                        ctx_h, ps_c[0:D, :], rzb[0:D, :], mybir.AluOpType.mult
                    )
                    ctx_tiles.append(ctx_h)

                    nc.sync.dma_start(out=attn_dst[h][:, :, qsl], in_=e_sb)

                # out projection partial for this q chunk
                o_sb = outp.tile([P, KO, NQ], F32, tag="out")
                for it in range(KO):
                    _heat()
                    ps_o = psum.tile([P, NQ], F32, tag="mm_out", bufs=2)
                    for h in range(HL):
                        nc.tensor.matmul(
                            ps_o,
                            lhsT=wo_sb[:, h, it * P : (it + 1) * P],
                            rhs=ctx_tiles[h],
                            start=(h == 0),
                            stop=(h == HL - 1),
                        )
                    nc.scalar.activation(
                        out=o_sb[:, it, :],
                        in_=ps_o,
                        func=mybir.ActivationFunctionType.Identity,
                        bias=bo_sb[:, it : it + 1],
                        scale=1.0,
                    )
                nc.sync.dma_start(out=out_dst[:, :, qsl], in_=o_sb)

    return nc


_NC_CACHE = None


def _get_nc():
    global _NC_CACHE
    if _NC_CACHE is None:
        _NC_CACHE = build_nc()
        _split_multi_waits(_NC_CACHE)
    return _NC_CACHE


def _core_inputs(c, query, key, value, task_bias, attention_mask,
                 Wq, bq, Wk, bk, Wv, bv, Wtb, btb, Wo, bo, xts):
    b = c // 4
    g = c % 4
    hs = g * HL                      # first head of this core
    fs = slice(hs * D, (hs + HL) * D)  # feature slice (192 wide)

    def bias2(vec):
        out = np.zeros((P, 2), np.float32)
        sl = np.asarray(vec[fs], np.float32)
        out[:, 0] = sl[0:P]
        out[: HL * D - P, 1] = sl[P:]
        return out

    bo_eff = np.asarray(bo, np.float32) if g == 0 else np.zeros((E,), np.float32)
    m = np.asarray(attention_mask[b], np.float32)
    maskbias = np.ascontiguousarray(
        ((m - 1.0) * MASK_NEG).reshape(KK, P).T
    ).astype(np.float32)

    wo_slice = np.asarray(Wo[:, fs], np.float32)          # [768, 192]
    wo_t = np.ascontiguousarray(
        wo_slice.T.reshape(HL, D, E).transpose(1, 0, 2)
    )                                                      # [64, 3, 768]

    return {
        "xq_t": xts[("q", b)],
        "xk_t": xts[("k", b)],
        "xv_t": xts[("v", b)],
        "wq_t": np.ascontiguousarray(np.asarray(Wq, np.float32)[fs].T),
        "wk_t": np.ascontiguousarray(np.asarray(Wk, np.float32)[fs].T),
        "wv_t": np.ascontiguousarray(np.asarray(Wv, np.float32)[fs].T),
        "wo_t": wo_t,
        "bq2": bias2(bq),
        "bk2": bias2(bk),
        "bv_row": np.ascontiguousarray(np.asarray(bv, np.float32)[fs]),
        "bo6": np.ascontiguousarray(bo_eff.reshape(KO, P).T),
        "task_b": np.asarray(task_bias[b], np.float32),
        "wtb_t": np.ascontiguousarray(np.asarray(Wtb, np.float32)[hs : hs + HL].T),
        "btb3": np.ascontiguousarray(np.asarray(btb, np.float32)[hs : hs + HL]),
        "maskbias": maskbias,
    }


def kernel(query, key, value, task_bias, attention_mask,
           Wq, bq, Wk, bk, Wv, bv, Wtb, btb, Wo, bo, **run_kwargs):
    query = np.asarray(query, np.float32)
    key = np.asarray(key, np.float32)
    value = np.asarray(value, np.float32)

    # host-side shard prep: transposed activations, shared across cores of a batch
    xts = {}
    for b in range(B):
        xts[("q", b)] = np.ascontiguousarray(query[b].T)
        xts[("k", b)] = np.ascontiguousarray(key[b].T)
        xts[("v", b)] = np.ascontiguousarray(value[b].T)

    in_maps = [
        _core_inputs(c, query, key, value, task_bias, attention_mask,
                     Wq, bq, Wk, bk, Wv, bv, Wtb, btb, Wo, bo, xts)
        for c in range(NCORES)
    ]

    nc = _get_nc()
    res = run_bass_kernel_spmd(nc, in_maps, list(range(NCORES)), **run_kwargs)
    results = res.results

    # gather attn: per-core [3, k, q] -> [B, H, k, q] -> transpose view [B,H,q,k]
    attn_kq = np.empty((B, H, S, S), np.float32)
    for c in range(NCORES):
        b, hs = c // 4, (c % 4) * HL
        attn_kq[b, hs : hs + HL] = results[c]["attn_t"]
    attn = attn_kq.transpose(0, 1, 3, 2)

    # gather out: sum the 4 partials per batch, transpose to [S, E]
    out = np.empty((B, S, E), np.float32)
    for b in range(B):
        acc = results[4 * b]["out_t"].copy()
        for g in range(1, 4):
            acc += results[4 * b + g]["out_t"]
        out[b] = acc.T

    if run_kwargs:
        kernel.last_result = res
    return out, attn
